# revision 2
# baseline (speedup 1.0000x reference)
"""MultiHeadAttention Trainium2 kernel (8 NeuronCores), v2.

Reference computation (torch-style Linear, x @ W.T):
    k = key @ W_k.T; v = value @ W_v.T; q = query (no projection)
    scores = q @ k.T / sqrt(64) per head; attn = softmax(scores)
    out = (attn @ v) @ W_o.T

Sharding: core = (batch b, head-group g), 4 heads per core; W_k/W_v
column-split by head, W_o partial-summed on host across the 4 groups.

v2 design (cost-model driven):
  - Everything bf16 on the PE (host converts); PSUM accumulation f32.
    Matmul cost = out_free_size x 1 cycle/row for bf16 at any width.
  - attn@V is TRANSPOSED: out[q,d] = ex[t,q].T @ v[t,d] with ex the
    stationary and v a 64-wide moving operand -> 64 cycles per
    (head, qblock, t-chunk) instead of 512: PE attnv cost halves.
    Softmax denominators via extra ones-column matmuls (ap=1).
  - Normalization becomes a per-partition tensor_scalar_mul (denom is
    per-q = per-partition); normalized [q,c] head-pairs are transposed
    on the PE (128x128, via identity) so W_o runs with a 128-deep
    contraction (half the instructions of per-head 64-deep).
  - exp on ScalarE is the hard floor (~33.5M elems/core * 0.833ns
    = 218us + per-instr overhead). All exp instrs are [128,1024]-wide
    to amortize the ~185ns PSUM/SBUF access overhead.
  - PSUM banks (bank-granular allocator): scores 2x[128,1024](4) +
    attnv accums 2x[128,512](2) + denoms [128,16](1) + proj/epi(1) = 8.
  - Phase 1 streams K/V projections while q-tile j0's attention chases
    the arriving chunks; phase 2 runs j1..j3 (ScalarE-bound) with each
    q-tile's W_o/transpose work interleaved into the NEXT tile's sweep.
"""

import os
import numpy as np
import ml_dtypes

import concourse.bacc as bacc
import concourse.tile as tile
import concourse.mybir as mybir
from concourse.bass_utils import run_bass_kernel_spmd

F32 = mybir.dt.float32
BF16 = mybir.dt.bfloat16
EXPF = mybir.ActivationFunctionType.Exp

B, NQ, NK, E, H, D = 2, 2048, 4096, 1024, 16, 64
HPC = 4          # heads per core
C = HPC * D      # head-channels per core (256)
TB = 256         # token block for streaming K/V projections
NTB = NK // TB   # 16
TCH = NK // 128  # 32 t-chunks
QT = 512         # q tile
NJ = NQ // QT    # 4

_last_results = None
_last_in_maps = None


def _build():
    nc = bacc.Bacc("TRN2", target_bir_lowering=False, debug=False, num_devices=8)

    keyT_d = nc.dram_tensor("keyT", [128, 8, NK], BF16, kind="ExternalInput").ap()
    valT_d = nc.dram_tensor("valT", [128, 8, NK], BF16, kind="ExternalInput").ap()
    qT_d = nc.dram_tensor("qT", [128, 2, NQ], BF16, kind="ExternalInput").ap()
    wkT_d = nc.dram_tensor("wkT", [128, 8, C], BF16, kind="ExternalInput").ap()
    wvT_d = nc.dram_tensor("wvT", [128, 8, C], BF16, kind="ExternalInput").ap()
    woT_d = nc.dram_tensor("woT", [128, 2, E], BF16, kind="ExternalInput").ap()
    ident_d = nc.dram_tensor("ident", [128, 128], BF16, kind="ExternalInput").ap()
    out_d = nc.dram_tensor("out", [NQ, E], F32, kind="ExternalOutput").ap()

    with tile.TileContext(nc) as tc:
        with (
            tc.tile_pool(name="wpool", bufs=1) as wpool,
            tc.tile_pool(name="stream", bufs=3) as stream,
            tc.tile_pool(name="expp", bufs=10) as expp,
            tc.tile_pool(name="epsb", bufs=4) as epsb,
            tc.tile_pool(name="outsb", bufs=2) as outsb,
            tc.tile_pool(name="pacc", bufs=1, space="PSUM") as pacc,
            tc.tile_pool(name="pscore", bufs=2, space="PSUM") as pscore,
        ):
            # ---- resident weights / q / identity / ones ----
            wk_sb = wpool.tile([128, 8, C], BF16)
            wv_sb = wpool.tile([128, 8, C], BF16)
            wo_sb = wpool.tile([128, 2, E], BF16)
            q_sb = wpool.tile([128, 2, NQ], BF16)
            id_sb = wpool.tile([128, 128], BF16)
            ones_sb = wpool.tile([128, 1], BF16)
            nc.vector.memset(ones_sb[:], 1.0)
            nc.sync.dma_start(wk_sb[:], wkT_d[:])

            # ---- resident kT / v ----
            kT_sb = wpool.tile([128, 2, NK], BF16)          # [c%128, c//128, t]
            vx_sb = wpool.tile([128, TCH, HPC, D], BF16)    # [t%128, t//128, h, d]

            def emit_scores(stile, p, t, q0, width):
                # stile[:, hh, :] = kT_h[:, tchunk].T @ q_h[:, q0:q0+width]
                for hh in range(2):
                    nc.tensor.matmul(
                        stile[:, hh, :],
                        kT_sb[hh * 64:(hh + 1) * 64, p, t * 128:(t + 1) * 128],
                        q_sb[hh * 64:(hh + 1) * 64, p, q0:q0 + width],
                        start=True, stop=True, tile_position=(hh * 64, 0))

            def emit_attnv(ex, acc, den, p, t):
                # acc[:, qb, hh, :] += ex[:, hh, qb*128:...].T @ v[:, t, h, :]
                # den[:, idx]       += ex_slice.T @ ones
                # PSUM start/stop are per 2KB zero-region (bank): exactly one
                # start (marks the whole bank pending-zero; first write of
                # each slice then overwrites) and one stop per bank per sweep.
                for hh in range(2):
                    h = 2 * p + hh
                    for qb in range(4):
                        exs = ex[:, hh, qb * 128:(qb + 1) * 128]
                        nc.tensor.matmul(acc[:, qb, hh, :], exs,
                                         vx_sb[:, t, h, :],
                                         start=(t == 0 and hh == 0 and qb == 0),
                                         stop=(t == TCH - 1 and hh == 1 and qb == 3),
                                         skip_group_check=True)
                        nc.tensor.matmul(den[:, p * 8 + qb * 2 + hh:
                                             p * 8 + qb * 2 + hh + 1],
                                         exs, ones_sb[:],
                                         start=(t == 0 and p == 0 and hh == 0 and qb == 0),
                                         stop=(t == TCH - 1 and p == 1 and hh == 1 and qb == 3),
                                         skip_group_check=True)

            def emit_norm_one(accs, rc, p, qb):
                # nm[128q, 128c] = acc * recip(denom) for one (p,qb); rc is
                # the fused [128,16] reciprocal tile of all denominators
                nm = epsb.tile([128, 128], BF16, tag="nm", bufs=8, name="nm")
                for hh in range(2):
                    idx = p * 8 + qb * 2 + hh
                    nc.vector.tensor_scalar_mul(
                        nm[:, hh * 64:(hh + 1) * 64],
                        accs[p][:, qb, hh, :], rc[:, idx:idx + 1])
                return nm

            def emit_recip(den):
                rc = epsb.tile([128, 16], F32, tag="rc", bufs=2, name="rc")
                nc.vector.reciprocal(rc[:], den[:])
                return rc

            def emit_norm(accs, den, nm_tiles):
                rc = emit_recip(den)
                for p in range(2):
                    for qb in range(4):
                        nm_tiles.append(emit_norm_one(accs, rc, p, qb))

            def emit_wo_piece(j, qb, k, nm_tiles, state, psum_pool, tag="epi"):
                # One small PE piece of a qb's W_o epilogue per call (k=0..3),
                # so the in-order PE queue never head-of-line blocks on the
                # DVE copies between transpose and W_o matmuls.
                if k in (0, 1):
                    p = k
                    tps = psum_pool.tile([128, 128], BF16, tag=tag, name="tps")
                    nc.tensor.matmul(tps[:], nm_tiles[p * 4 + qb][:], id_sb[:],
                                     is_transpose=True)
                    nt = epsb.tile([128, 128], BF16, tag="nt", bufs=4, name="nt")
                    nc.vector.tensor_copy(nt[:], tps[:])
                    state.setdefault((j, qb), [None, None, None])[p] = nt
                    return
                st = state[(j, qb)]
                if st[2] is None:
                    st[2] = outsb.tile([128, E], F32, tag="osb", name="osb")
                et = k - 2
                osb = st[2]
                wps = psum_pool.tile([128, QT], F32, tag=tag, name="wps")
                for p in range(2):
                    nc.tensor.matmul(wps[:], st[p][:],
                                     wo_sb[:, p, et * QT:(et + 1) * QT],
                                     start=(p == 0), stop=(p == 1))
                nc.vector.tensor_copy(osb[:, et * QT:(et + 1) * QT], wps[:])
                if et == 1:
                    q0 = j * QT
                    nc.sync.dma_start(
                        out_d[q0 + qb * 128:q0 + (qb + 1) * 128, :], osb[:])

            def emit_wo_qb(j, qb, nm_tiles, psum_pool, tag="epi"):
                state = {}
                for k in range(4):
                    emit_wo_piece(j, qb, k, nm_tiles, state, psum_pool, tag)

            # ================= PHASE 1: stream + projections + j0 =================
            acc_j = {0: (pacc.tile([128, 4, 2, D], F32, tag="accA", name="accA"),
                         pacc.tile([128, 4, 2, D], F32, tag="accB", name="accB"))}
            den_j = {0: pacc.tile([128, 16], F32, tag="den", name="den")}
            norm_tiles = {}

            with tc.tile_pool(name="pproj", bufs=1, space="PSUM") as pproj:
                ex_pend = {}
                for tb in range(NTB):
                    kblk = stream.tile([128, 8, TB], BF16, tag="kblk", name="kblk")
                    vblk = stream.tile([128, 8, TB], BF16, tag="vblk", name="vblk")
                    ts0 = tb * TB
                    nc.sync.dma_start(kblk[:], keyT_d[:, :, ts0:ts0 + TB])
                    if tb == 0:
                        nc.sync.dma_start(q_sb[:, :, 0:QT], qT_d[:, :, 0:QT])
                    nc.sync.dma_start(vblk[:], valT_d[:, :, ts0:ts0 + TB])
                    if tb == 0:
                        nc.sync.dma_start(wv_sb[:], wvT_d[:])
                        nc.sync.dma_start(q_sb[:, :, QT:NQ], qT_d[:, :, QT:NQ])
                        nc.sync.dma_start(id_sb[:], ident_d[:])
                    def sweep_scores(t):
                        exs = {}
                        for p in range(2):
                            stile = pscore.tile([128, 2, QT], F32, tag="sc", name="st")
                            emit_scores(stile, p, t, 0, QT)
                            ex = expp.tile([128, 2, QT], BF16, tag="exp", name="ex")
                            nc.scalar.activation(ex[:], stile[:], EXPF, scale=0.125)
                            exs[p] = ex
                        return exs

                    def sweep_attnv(t, exs):
                        for p in range(2):
                            emit_attnv(exs[p], acc_j[0][p], den_j[0], p, t)

                    # K projection: kT[c, t] over this block
                    for mc in range(2):
                        kps = pproj.tile([128, TB], F32, tag="p", name="kps")
                        for cc in range(8):
                            nc.tensor.matmul(kps[:],
                                             wk_sb[:, cc, mc * 128:(mc + 1) * 128],
                                             kblk[:, cc, :],
                                             start=(cc == 0), stop=(cc == 7))
                        nc.vector.tensor_copy(kT_sb[:, mc, ts0:ts0 + TB], kps[:])
                    # V projection: v[t, c] over this block
                    for t2 in range(TB // 128):
                        vps = pproj.tile([128, C], F32, tag="p", name="vps")
                        for cc in range(8):
                            nc.tensor.matmul(vps[:],
                                             vblk[:, cc, t2 * 128:(t2 + 1) * 128],
                                             wv_sb[:, cc, :],
                                             start=(cc == 0), stop=(cc == 7))
                        tg = tb * (TB // 128) + t2
                        nc.vector.tensor_copy(
                            vx_sb[:, tg, :, :],
                            vps[:].rearrange("p (h d) -> p h d", h=HPC))
                    # score THIS block's chunks (kT-copy round trip hides
                    # behind V proj); attn@V lags one block
                    ex_pend[2 * tb] = sweep_scores(2 * tb)
                    ex_pend[2 * tb + 1] = sweep_scores(2 * tb + 1)
                    if tb > 0:
                        sweep_attnv(2 * (tb - 1), ex_pend.pop(2 * (tb - 1)))
                        sweep_attnv(2 * (tb - 1) + 1, ex_pend.pop(2 * (tb - 1) + 1))
                for t in (TCH - 2, TCH - 1):
                    sweep_attnv(t, ex_pend.pop(t))
                nc.sync.dma_start(wo_sb[:], woT_d[:])

            # ================= PHASE 2: j1..j3 + all epilogues/W_o =================
            with tc.tile_pool(name="pepi", bufs=1, space="PSUM") as pepi:
                wo_state = {}
                for j in range(1, NJ):
                    q0 = j * QT
                    acc_j[j] = (pacc.tile([128, 4, 2, D], F32, tag="accA", name="accA"),
                                pacc.tile([128, 4, 2, D], F32, tag="accB", name="accB"))
                    den_j[j] = pacc.tile([128, 16], F32, tag="den", name="den")
                    # attn@V lags the exp stream by LAG chunks so the PE's
                    # in-order queue never holds the next scores behind an
                    # attnv that is waiting on the previous tile's epilogue
                    LAG = 2
                    exq = {}
                    for t in range(TCH + LAG):
                        if t < TCH:
                            exs = {}
                            for p in range(2):
                                stile = pscore.tile([128, 2, QT], F32, tag="sc", name="st")
                                emit_scores(stile, p, t, q0, QT)
                                ex = expp.tile([128, 2, QT], BF16, tag="exp", name="ex")
                                nc.scalar.activation(ex[:], stile[:], EXPF, scale=0.125)
                                exs[p] = ex
                            exq[t] = exs
                        if t == 0:
                            # previous q-tile's normalize AFTER this tile's
                            # first scores+exp are queued: the acc/den WAR
                            # drain overlaps the exp stream instead of
                            # stalling it at the boundary
                            norm_tiles[j - 1] = []
                            emit_norm(acc_j[j - 1], den_j[j - 1], norm_tiles[j - 1])
                        if t >= LAG:
                            ta = t - LAG
                            exa = exq.pop(ta)
                            for p in range(2):
                                emit_attnv(exa[p], acc_j[j][p],
                                           den_j[j], p, ta)
                        # interleave previous q-tile's W_o work into this
                        # sweep, one small PE piece per chunk
                        if 2 <= t < 18:
                            qb, k = (t - 2) // 4, (t - 2) % 4
                            emit_wo_piece(j - 1, qb, k, norm_tiles[j - 1],
                                          wo_state, pepi)
                    norm_tiles[j] = []
                # tail: last q-tile's epilogue + W_o, interleaved per qb so
                # each W_o starts as soon as its two nm tiles exist. The score
                # ring (4 banks) is idle after the last exp — run the tail W_o
                # tiles through it (same tag -> 2 ring slots) so transposes
                # and W_o overlap instead of serializing on pepi's one bank.
                rc_last = emit_recip(den_j[NJ - 1])
                nm_last = [None] * 8
                for qb in range(4):
                    for p in range(2):
                        nm_last[p * 4 + qb] = emit_norm_one(
                            acc_j[NJ - 1], rc_last, p, qb)
                    emit_wo_qb(NJ - 1, qb, nm_last, pscore, tag="sc")

    nc.compile()
    return nc


_nc = None


def kernel(query, key, value, W_k, W_v, W_o):
    global _nc, _last_results, _last_in_maps
    if _nc is None:
        _nc = _build()

    bf = ml_dtypes.bfloat16
    query = np.asarray(query, dtype=np.float32)
    key = np.asarray(key, dtype=np.float32)
    value = np.asarray(value, dtype=np.float32)
    W_k = np.asarray(W_k, dtype=np.float32)
    W_v = np.asarray(W_v, dtype=np.float32)
    W_o = np.asarray(W_o, dtype=np.float32)

    def part3(a, inner):
        # [R, N] -> [128, R//128, N] with partition = row % 128
        r, n = a.shape
        return np.ascontiguousarray(
            a.reshape(r // 128, 128, n).transpose(1, 0, 2).astype(inner))

    keyT = [part3(key[b].T, bf) for b in range(B)]
    valT = [part3(value[b].T, bf) for b in range(B)]
    ident = np.eye(128, dtype=bf)

    in_maps = []
    for b in range(B):
        for g in range(4):
            c0 = g * C
            in_maps.append({
                "keyT": keyT[b],
                "valT": valT[b],
                "qT": part3(np.ascontiguousarray(query[b][:, c0:c0 + C].T), bf),
                "wkT": part3(np.ascontiguousarray(W_k[c0:c0 + C, :].T), bf),
                "wvT": part3(np.ascontiguousarray(W_v[c0:c0 + C, :].T), bf),
                "woT": part3(np.ascontiguousarray(W_o[:, c0:c0 + C].T), bf),
                "ident": ident,
            })

    _last_in_maps = in_maps
    res = run_bass_kernel_spmd(
        _nc, in_maps, core_ids=list(range(8)),
        trace=bool(os.environ.get("BASS_TRACE")))
    _last_results = res

    out = np.zeros((B, NQ, E), dtype=np.float32)
    for b in range(B):
        for g in range(4):
            out[b] += res.results[b * 4 + g]["out"]
    return out


# revision 3
# speedup vs baseline: 1.0008x; 1.0008x over previous
"""MultiHeadAttention Trainium2 kernel (8 NeuronCores), v2.

Reference computation (torch-style Linear, x @ W.T):
    k = key @ W_k.T; v = value @ W_v.T; q = query (no projection)
    scores = q @ k.T / sqrt(64) per head; attn = softmax(scores)
    out = (attn @ v) @ W_o.T

Sharding: core = (batch b, head-group g), 4 heads per core; W_k/W_v
column-split by head, W_o partial-summed on host across the 4 groups.

v2 design (cost-model driven):
  - Everything bf16 on the PE (host converts); PSUM accumulation f32.
    Matmul cost = out_free_size x 1 cycle/row for bf16 at any width.
  - attn@V is TRANSPOSED: out[q,d] = ex[t,q].T @ v[t,d] with ex the
    stationary and v a 64-wide moving operand -> 64 cycles per
    (head, qblock, t-chunk) instead of 512: PE attnv cost halves.
    Softmax denominators via extra ones-column matmuls (ap=1).
  - Normalization becomes a per-partition tensor_scalar_mul (denom is
    per-q = per-partition); normalized [q,c] head-pairs are transposed
    on the PE (128x128, via identity) so W_o runs with a 128-deep
    contraction (half the instructions of per-head 64-deep).
  - exp on ScalarE is the hard floor (~33.5M elems/core * 0.833ns
    = 218us + per-instr overhead). All exp instrs are [128,1024]-wide
    to amortize the ~185ns PSUM/SBUF access overhead.
  - PSUM banks (bank-granular allocator): scores 2x[128,1024](4) +
    attnv accums 2x[128,512](2) + denoms [128,16](1) + proj/epi(1) = 8.
  - Phase 1 streams K/V projections while q-tile j0's attention chases
    the arriving chunks; phase 2 runs j1..j3 (ScalarE-bound) with each
    q-tile's W_o/transpose work interleaved into the NEXT tile's sweep.
"""

import os
import numpy as np
import ml_dtypes

import concourse.bacc as bacc
import concourse.tile as tile
import concourse.mybir as mybir
from concourse.bass_utils import run_bass_kernel_spmd

F32 = mybir.dt.float32
BF16 = mybir.dt.bfloat16
EXPF = mybir.ActivationFunctionType.Exp

B, NQ, NK, E, H, D = 2, 2048, 4096, 1024, 16, 64
HPC = 4          # heads per core
C = HPC * D      # head-channels per core (256)
TB = 256         # token block for streaming K/V projections
NTB = NK // TB   # 16
TCH = NK // 128  # 32 t-chunks
QT = 512         # q tile
NJ = NQ // QT    # 4

_last_results = None
_last_in_maps = None


def _build():
    nc = bacc.Bacc("TRN2", target_bir_lowering=False, debug=False, num_devices=8)

    keyT_d = nc.dram_tensor("keyT", [128, 8, NK], BF16, kind="ExternalInput").ap()
    valT_d = nc.dram_tensor("valT", [128, 8, NK], BF16, kind="ExternalInput").ap()
    qT_d = nc.dram_tensor("qT", [128, 2, NQ], BF16, kind="ExternalInput").ap()
    wkT_d = nc.dram_tensor("wkT", [128, 8, C], BF16, kind="ExternalInput").ap()
    wvT_d = nc.dram_tensor("wvT", [128, 8, C], BF16, kind="ExternalInput").ap()
    woT_d = nc.dram_tensor("woT", [128, 2, E], BF16, kind="ExternalInput").ap()
    ident_d = nc.dram_tensor("ident", [128, 128], BF16, kind="ExternalInput").ap()
    out_d = nc.dram_tensor("out", [NQ, E], F32, kind="ExternalOutput").ap()

    with tile.TileContext(nc) as tc:
        with (
            tc.tile_pool(name="wpool", bufs=1) as wpool,
            tc.tile_pool(name="stream", bufs=3) as stream,
            tc.tile_pool(name="expp", bufs=34) as expp,
            tc.tile_pool(name="epsb", bufs=4) as epsb,
            tc.tile_pool(name="outsb", bufs=2) as outsb,
            tc.tile_pool(name="pacc", bufs=1, space="PSUM") as pacc,
            tc.tile_pool(name="pscore", bufs=2, space="PSUM") as pscore,
        ):
            # ---- resident weights / q / identity / ones ----
            wk_sb = wpool.tile([128, 8, C], BF16)
            wv_sb = wpool.tile([128, 8, C], BF16)
            wo_sb = wpool.tile([128, 2, E], BF16)
            q_sb = wpool.tile([128, 2, NQ], BF16)
            id_sb = wpool.tile([128, 128], BF16)
            ones_sb = wpool.tile([128, 1], BF16)
            nc.vector.memset(ones_sb[:], 1.0)
            nc.sync.dma_start(wk_sb[:], wkT_d[:])

            # ---- resident kT / v ----
            kT_sb = wpool.tile([128, 2, NK], BF16)          # [c%128, c//128, t]
            vx_sb = wpool.tile([128, TCH, HPC, D], BF16)    # [t%128, t//128, h, d]

            def emit_scores(stile, p, t, q0, width):
                # stile[:, hh, :] = kT_h[:, tchunk].T @ q_h[:, q0:q0+width]
                for hh in range(2):
                    nc.tensor.matmul(
                        stile[:, hh, :],
                        kT_sb[hh * 64:(hh + 1) * 64, p, t * 128:(t + 1) * 128],
                        q_sb[hh * 64:(hh + 1) * 64, p, q0:q0 + width],
                        start=True, stop=True, tile_position=(hh * 64, 0))

            def emit_attnv(ex, acc, den, p, t):
                # acc[:, qb, hh, :] += ex[:, hh, qb*128:...].T @ v[:, t, h, :]
                # den[:, idx]       += ex_slice.T @ ones
                # PSUM start/stop are per 2KB zero-region (bank): exactly one
                # start (marks the whole bank pending-zero; first write of
                # each slice then overwrites) and one stop per bank per sweep.
                for hh in range(2):
                    h = 2 * p + hh
                    for qb in range(4):
                        exs = ex[:, hh, qb * 128:(qb + 1) * 128]
                        nc.tensor.matmul(acc[:, qb, hh, :], exs,
                                         vx_sb[:, t, h, :],
                                         start=(t == 0 and hh == 0 and qb == 0),
                                         stop=(t == TCH - 1 and hh == 1 and qb == 3),
                                         skip_group_check=True)
                        nc.tensor.matmul(den[:, p * 8 + qb * 2 + hh:
                                             p * 8 + qb * 2 + hh + 1],
                                         exs, ones_sb[:],
                                         start=(t == 0 and p == 0 and hh == 0 and qb == 0),
                                         stop=(t == TCH - 1 and p == 1 and hh == 1 and qb == 3),
                                         skip_group_check=True)

            def emit_norm_one(accs, rc, p, qb):
                # nm[128q, 128c] = acc * recip(denom) for one (p,qb); rc is
                # the fused [128,16] reciprocal tile of all denominators
                nm = epsb.tile([128, 128], BF16, tag="nm", bufs=8, name="nm")
                for hh in range(2):
                    idx = p * 8 + qb * 2 + hh
                    nc.vector.tensor_scalar_mul(
                        nm[:, hh * 64:(hh + 1) * 64],
                        accs[p][:, qb, hh, :], rc[:, idx:idx + 1])
                return nm

            def emit_recip(den):
                rc = epsb.tile([128, 16], F32, tag="rc", bufs=2, name="rc")
                nc.vector.reciprocal(rc[:], den[:])
                return rc

            def emit_norm(accs, den, nm_tiles):
                rc = emit_recip(den)
                for p in range(2):
                    for qb in range(4):
                        nm_tiles.append(emit_norm_one(accs, rc, p, qb))

            def emit_wo_piece(j, qb, k, nm_tiles, state, psum_pool, tag="epi"):
                # One small PE piece of a qb's W_o epilogue per call (k=0..3),
                # so the in-order PE queue never head-of-line blocks on the
                # DVE copies between transpose and W_o matmuls.
                if k in (0, 1):
                    p = k
                    tps = psum_pool.tile([128, 128], BF16, tag=tag, name="tps")
                    nc.tensor.matmul(tps[:], nm_tiles[p * 4 + qb][:], id_sb[:],
                                     is_transpose=True)
                    nt = epsb.tile([128, 128], BF16, tag="nt", bufs=4, name="nt")
                    nc.vector.tensor_copy(nt[:], tps[:])
                    state.setdefault((j, qb), [None, None, None])[p] = nt
                    return
                st = state[(j, qb)]
                if st[2] is None:
                    st[2] = outsb.tile([128, E], F32, tag="osb", name="osb")
                et = k - 2
                osb = st[2]
                wps = psum_pool.tile([128, QT], F32, tag=tag, name="wps")
                for p in range(2):
                    nc.tensor.matmul(wps[:], st[p][:],
                                     wo_sb[:, p, et * QT:(et + 1) * QT],
                                     start=(p == 0), stop=(p == 1))
                nc.vector.tensor_copy(osb[:, et * QT:(et + 1) * QT], wps[:])
                if et == 1:
                    q0 = j * QT
                    nc.sync.dma_start(
                        out_d[q0 + qb * 128:q0 + (qb + 1) * 128, :], osb[:])

            def emit_wo_qb(j, qb, nm_tiles, psum_pool, tag="epi"):
                state = {}
                for k in range(4):
                    emit_wo_piece(j, qb, k, nm_tiles, state, psum_pool, tag)

            # ================= PHASE 1: stream + projections + j0 =================
            acc_j = {0: (pacc.tile([128, 4, 2, D], F32, tag="accA", name="accA"),
                         pacc.tile([128, 4, 2, D], F32, tag="accB", name="accB"))}
            den_j = {0: pacc.tile([128, 16], F32, tag="den", name="den")}
            norm_tiles = {}

            with tc.tile_pool(name="pproj", bufs=1, space="PSUM") as pproj:
                ex_pend = {}
                for tb in range(NTB):
                    kblk = stream.tile([128, 8, TB], BF16, tag="kblk", name="kblk")
                    vblk = stream.tile([128, 8, TB], BF16, tag="vblk", name="vblk")
                    ts0 = tb * TB
                    nc.sync.dma_start(kblk[:], keyT_d[:, :, ts0:ts0 + TB])
                    if tb == 0:
                        nc.sync.dma_start(q_sb[:, :, 0:QT], qT_d[:, :, 0:QT])
                    nc.sync.dma_start(vblk[:], valT_d[:, :, ts0:ts0 + TB])
                    if tb == 0:
                        nc.sync.dma_start(wv_sb[:], wvT_d[:])
                        nc.sync.dma_start(q_sb[:, :, QT:NQ], qT_d[:, :, QT:NQ])
                        nc.sync.dma_start(id_sb[:], ident_d[:])
                    def sweep_scores(t):
                        exs = {}
                        for p in range(2):
                            stile = pscore.tile([128, 2, QT], F32, tag="sc", name="st")
                            emit_scores(stile, p, t, 0, QT)
                            ex = expp.tile([128, 2, QT], BF16, tag="exp", name="ex")
                            nc.scalar.activation(ex[:], stile[:], EXPF, scale=0.125)
                            exs[p] = ex
                        return exs

                    def sweep_attnv(t, exs):
                        for p in range(2):
                            emit_attnv(exs[p], acc_j[0][p], den_j[0], p, t)

                    # K projection: kT[c, t] over this block
                    for mc in range(2):
                        kps = pproj.tile([128, TB], F32, tag="p", name="kps")
                        for cc in range(8):
                            nc.tensor.matmul(kps[:],
                                             wk_sb[:, cc, mc * 128:(mc + 1) * 128],
                                             kblk[:, cc, :],
                                             start=(cc == 0), stop=(cc == 7))
                        nc.vector.tensor_copy(kT_sb[:, mc, ts0:ts0 + TB], kps[:])
                    # V projection: v[t, c] over this block
                    for t2 in range(TB // 128):
                        vps = pproj.tile([128, C], F32, tag="p", name="vps")
                        for cc in range(8):
                            nc.tensor.matmul(vps[:],
                                             vblk[:, cc, t2 * 128:(t2 + 1) * 128],
                                             wv_sb[:, cc, :],
                                             start=(cc == 0), stop=(cc == 7))
                        tg = tb * (TB // 128) + t2
                        nc.vector.tensor_copy(
                            vx_sb[:, tg, :, :],
                            vps[:].rearrange("p (h d) -> p h d", h=HPC))
                    # score THIS block's chunks (kT-copy round trip hides
                    # behind V proj); attn@V lags one block
                    ex_pend[2 * tb] = sweep_scores(2 * tb)
                    ex_pend[2 * tb + 1] = sweep_scores(2 * tb + 1)
                    if tb > 0:
                        for t in (2 * (tb - 1), 2 * (tb - 1) + 1):
                            if t < TCH - 8:
                                sweep_attnv(t, ex_pend.pop(t))
                # j0's last 8 chunks' attn@V stay buffered in ex_pend and are
                # drained inside j1's sweep where the PE has slack
                nc.sync.dma_start(wo_sb[:], woT_d[:])

            # ================= PHASE 2: j1..j3 + all epilogues/W_o =================
            with tc.tile_pool(name="pepi", bufs=1, space="PSUM") as pepi:
                wo_state = {}
                for j in range(1, NJ):
                    q0 = j * QT
                    acc_j[j] = (pacc.tile([128, 4, 2, D], F32, tag="accA", name="accA"),
                                pacc.tile([128, 4, 2, D], F32, tag="accB", name="accB"))
                    den_j[j] = pacc.tile([128, 16], F32, tag="den", name="den")
                    # attn@V lags the exp stream by LAG chunks so the PE's
                    # in-order queue never holds the next scores behind an
                    # attnv that is waiting on the previous tile's epilogue.
                    # For j==1 the lag is 8: j0's deferred last-8 attn@V
                    # chunks drain during chunks 0..7, then j0 normalizes and
                    # j1's own attn@V catches up (2 chunks every 3).
                    LAG = 8 if j == 1 else 2
                    NRM = 8 if j == 1 else 0
                    WO0 = NRM + 2
                    exq = {}
                    done = 0
                    for t in range(TCH + LAG):
                        if t < TCH:
                            exs = {}
                            for p in range(2):
                                stile = pscore.tile([128, 2, QT], F32, tag="sc", name="st")
                                emit_scores(stile, p, t, q0, QT)
                                ex = expp.tile([128, 2, QT], BF16, tag="exp", name="ex")
                                nc.scalar.activation(ex[:], stile[:], EXPF, scale=0.125)
                                exs[p] = ex
                            exq[t] = exs
                        if j == 1 and t < 8:
                            td = TCH - 8 + t
                            sweep_attnv(td, ex_pend.pop(td))
                        if t == NRM:
                            # previous q-tile's normalize AFTER this tile's
                            # scores+exp are queued: the acc/den WAR drain
                            # overlaps the exp stream
                            norm_tiles[j - 1] = []
                            emit_norm(acc_j[j - 1], den_j[j - 1], norm_tiles[j - 1])
                        if t >= LAG:
                            navail = t - LAG + 1
                            ncatch = min(TCH, navail + (navail // 3 if j == 1 else 0))
                            while done < ncatch:
                                exa = exq.pop(done)
                                for p in range(2):
                                    emit_attnv(exa[p], acc_j[j][p],
                                               den_j[j], p, done)
                                done += 1
                        # interleave previous q-tile's W_o work into this
                        # sweep, one small PE piece per chunk
                        if WO0 <= t < WO0 + 16:
                            qb, k = (t - WO0) // 4, (t - WO0) % 4
                            emit_wo_piece(j - 1, qb, k, norm_tiles[j - 1],
                                          wo_state, pepi)
                    while done < TCH:
                        exa = exq.pop(done)
                        for p in range(2):
                            emit_attnv(exa[p], acc_j[j][p], den_j[j], p, done)
                        done += 1
                    norm_tiles[j] = []
                # tail: last q-tile's epilogue + W_o, interleaved per qb so
                # each W_o starts as soon as its two nm tiles exist. The score
                # ring (4 banks) is idle after the last exp — run the tail W_o
                # tiles through it (same tag -> 2 ring slots) so transposes
                # and W_o overlap instead of serializing on pepi's one bank.
                rc_last = emit_recip(den_j[NJ - 1])
                nm_last = [None] * 8
                for qb in range(4):
                    for p in range(2):
                        nm_last[p * 4 + qb] = emit_norm_one(
                            acc_j[NJ - 1], rc_last, p, qb)
                    emit_wo_qb(NJ - 1, qb, nm_last, pscore, tag="sc")

    nc.compile()
    return nc


_nc = None


def kernel(query, key, value, W_k, W_v, W_o):
    global _nc, _last_results, _last_in_maps
    if _nc is None:
        _nc = _build()

    bf = ml_dtypes.bfloat16
    query = np.asarray(query, dtype=np.float32)
    key = np.asarray(key, dtype=np.float32)
    value = np.asarray(value, dtype=np.float32)
    W_k = np.asarray(W_k, dtype=np.float32)
    W_v = np.asarray(W_v, dtype=np.float32)
    W_o = np.asarray(W_o, dtype=np.float32)

    def part3(a, inner):
        # [R, N] -> [128, R//128, N] with partition = row % 128
        r, n = a.shape
        return np.ascontiguousarray(
            a.reshape(r // 128, 128, n).transpose(1, 0, 2).astype(inner))

    keyT = [part3(key[b].T, bf) for b in range(B)]
    valT = [part3(value[b].T, bf) for b in range(B)]
    ident = np.eye(128, dtype=bf)

    in_maps = []
    for b in range(B):
        for g in range(4):
            c0 = g * C
            in_maps.append({
                "keyT": keyT[b],
                "valT": valT[b],
                "qT": part3(np.ascontiguousarray(query[b][:, c0:c0 + C].T), bf),
                "wkT": part3(np.ascontiguousarray(W_k[c0:c0 + C, :].T), bf),
                "wvT": part3(np.ascontiguousarray(W_v[c0:c0 + C, :].T), bf),
                "woT": part3(np.ascontiguousarray(W_o[:, c0:c0 + C].T), bf),
                "ident": ident,
            })

    _last_in_maps = in_maps
    res = run_bass_kernel_spmd(
        _nc, in_maps, core_ids=list(range(8)),
        trace=bool(os.environ.get("BASS_TRACE")))
    _last_results = res

    out = np.zeros((B, NQ, E), dtype=np.float32)
    for b in range(B):
        for g in range(4):
            out[b] += res.results[b * 4 + g]["out"]
    return out
